# revision 3
# baseline (speedup 1.0000x reference)
"""Trainium2 Bass kernel for nn_MixtureOfExpertsLoss.

Data-parallel over tokens across 8 NeuronCores (1024 tokens/core). Per core:
  - stream logits [1024, 32000] f32 (131 MB) through SBUF in [128, 8000]
    chunks via HWDGE DMA; each chunk gets a fused Exp + per-partition row-sum
    (ACT accum_out), so per-token sum(exp(x)) falls out of the streaming pass
    with no extra reduce traffic. Kernel is HBM-bandwidth-bound (the roofline
    for this problem).
  - label logits fetched with an indirect-DMA element gather (offsets
    precomputed on host: t*V + label[t]).
  - logZ = Ln(sumexp); per-token CE contribution = (logZ - logit[label]) *
    (label != 0).
  - gate softmax load vector and expert-index histogram (size E=8) on DVE.
  - per-core partials written out as one [128, 32] f32 stats tile.
Host: sums the 8 stats tiles (the size-E "all-reduce" + CE sum/count from the
sharding hint) and finishes the ~50-flop variance/scalar combine.
"""

import numpy as np

import concourse.bass as bass
import concourse.tile as tile
from concourse import mybir
from concourse.bass_utils import run_bass_kernel_spmd

AUX_W = 0.01
LB_W = 0.01
IGNORE_INDEX = 0

B, S, V, E, K = 4, 2048, 32000, 8, 2
N_CORES = 8
NT = B * S            # 8192 tokens total
TPC = NT // N_CORES   # 1024 tokens per core
P = 128               # partitions
NB = TPC // P         # 8 token blocks per core
F = 8000              # vocab chunk (free dim) per DMA/ACT op
NCH = V // F          # 4 chunks per block

F32 = mybir.dt.float32
I32 = mybir.dt.int32

_nc_cache = None
_last_results = None
_wsplit_counter = [0]


def _split_multiwait(nc, max_waits=1):
    """Hoist extra semaphore waits onto standalone EventSemaphore instructions.

    The static-DMA walrus lowering here supports only one sync-wait command
    per instruction (Tile's kernel-tail drain otherwise fails codegen with
    "Too many sync wait commands"). Inserting the extra waits immediately
    before the offender on the same engine preserves semantics exactly.
    """
    n = 0
    for fn in nc.m.functions:
        for bb in fn.blocks:
            out = []
            changed = False
            for inst in bb.instructions:
                si = inst.sync_info
                if si is not None and len(si.on_wait) > max_waits:
                    waits = list(si.on_wait)
                    for w in waits[:-max_waits]:
                        _wsplit_counter[0] += 1
                        out.append(
                            mybir.InstEventSemaphore(
                                name=f"wsplit_{_wsplit_counter[0]}",
                                engine=inst.engine,
                                ins=[],
                                outs=[],
                                sync_info=mybir.SyncInfo(on_wait=[w], on_update=[]),
                            )
                        )
                        n += 1
                    inst.sync_info = mybir.SyncInfo(
                        on_wait=waits[-max_waits:], on_update=list(si.on_update)
                    )
                    changed = True
                out.append(inst)
            if changed:
                bb.instructions = out
    return n


def _build():
    nc = bass.Bass()
    lg = nc.dram_tensor("logits", [TPC, V], F32, kind="ExternalInput")
    goff = nc.dram_tensor("goff", [P, NB], I32, kind="ExternalInput")
    labf = nc.dram_tensor("labf", [P, NB], F32, kind="ExternalInput")
    gate = nc.dram_tensor("gate", [P, NB * E], F32, kind="ExternalInput")
    eidx = nc.dram_tensor("eidx", [P, NB * K], F32, kind="ExternalInput")
    stats_d = nc.dram_tensor("stats", [P, 32], F32, kind="ExternalOutput")

    lg2 = lg[:, :]
    lg_flat = lg2.rearrange("t v -> (t v)").unsqueeze(1)  # [TPC*V, 1] for gather

    Exp = mybir.ActivationFunctionType.Exp
    Ln = mybir.ActivationFunctionType.Ln
    Op = mybir.AluOpType
    AX = mybir.AxisListType.X

    with tile.TileContext(nc) as tc:
        with (
            tc.tile_pool(name="io", bufs=3) as io,
            tc.tile_pool(name="scratch", bufs=1) as scratch,
            tc.tile_pool(name="small", bufs=1) as small,
        ):
            goff_t = small.tile([P, NB], I32)
            nc.sync.dma_start(out=goff_t[:], in_=goff[:, :])
            labf_t = small.tile([P, NB], F32)
            nc.sync.dma_start(out=labf_t[:], in_=labf[:, :])
            gate_t = small.tile([P, NB * E], F32)
            nc.sync.dma_start(out=gate_t[:], in_=gate[:, :])
            eidx_t = small.tile([P, NB * K], F32)
            nc.sync.dma_start(out=eidx_t[:], in_=eidx[:, :])

            # gate exp early: same ACT table set as the streaming Exp ops
            gexp = small.tile([P, NB * E], F32)
            nc.scalar.activation(out=gexp[:], in_=gate_t[:], func=Exp)

            # label-logit gather (one element per token)
            ll = small.tile([P, NB], F32)
            for b in range(NB):
                nc.gpsimd.indirect_dma_start(
                    out=ll[:, b : b + 1],
                    out_offset=None,
                    in_=lg_flat,
                    in_offset=bass.IndirectOffsetOnAxis(
                        ap=goff_t[:, b : b + 1], axis=0
                    ),
                )

            # hot loop: stream logits, fused exp + row-sum accumulate
            sums = small.tile([P, NB, NCH], F32)
            xexp = scratch.tile([P, F], F32)  # exp output, never read back
            for b in range(NB):
                for c in range(NCH):
                    xt = io.tile([P, F], F32, tag="xt")
                    nc.sync.dma_start(
                        out=xt[:],
                        in_=lg2[b * P : (b + 1) * P, c * F : (c + 1) * F],
                    )
                    nc.scalar.activation(
                        out=xexp[:],
                        in_=xt[:],
                        func=Exp,
                        accum_out=sums[:, b, c : c + 1],
                    )

            sumexp = small.tile([P, NB], F32)
            nc.vector.reduce_sum(out=sumexp[:], in_=sums[:, :, :], axis=AX)
            logz = small.tile([P, NB], F32)
            nc.scalar.activation(out=logz[:], in_=sumexp[:], func=Ln)

            stats = small.tile([P, 32], F32)
            # valid mask: cols 8:16 = (label != 0)
            inv = small.tile([P, NB], F32)
            nc.vector.tensor_scalar(
                out=inv[:], in0=labf_t[:], scalar1=0.0, scalar2=None, op0=Op.is_equal
            )
            nc.vector.tensor_scalar(
                out=stats[:, 8:16], in0=inv[:], scalar1=-1.0, scalar2=1.0,
                op0=Op.mult, op1=Op.add,
            )
            # cols 0:8 = (logZ - logit[label]) * valid
            nll = small.tile([P, NB], F32)
            nc.vector.tensor_tensor(out=nll[:], in0=logz[:], in1=ll[:], op=Op.subtract)
            nc.vector.tensor_tensor(
                out=stats[:, 0:8], in0=nll[:], in1=stats[:, 8:16], op=Op.mult
            )

            # cols 16:24 = per-expert gate-prob load partials
            gv = gexp[:].rearrange("p (b e) -> p b e", e=E)
            gsum = small.tile([P, NB], F32)
            nc.vector.reduce_sum(out=gsum[:], in_=gv, axis=AX)
            grec = small.tile([P, NB], F32)
            nc.vector.reciprocal(out=grec[:], in_=gsum[:])
            gtmp = small.tile([P, NB], F32)
            for e in range(E):
                nc.vector.tensor_tensor(
                    out=gtmp[:], in0=gv[:, :, e], in1=grec[:], op=Op.mult
                )
                nc.vector.reduce_sum(
                    out=stats[:, 16 + e : 17 + e], in_=gtmp[:], axis=AX
                )

            # cols 24:32 = expert-index histogram partials
            ctmp = small.tile([P, NB * K], F32)
            for e in range(E):
                nc.vector.tensor_scalar(
                    out=ctmp[:], in0=eidx_t[:], scalar1=float(e), scalar2=0.0,
                    op0=Op.is_equal, op1=Op.add,
                    accum_out=stats[:, 24 + e : 25 + e],
                )

            nc.gpsimd.dma_start(out=stats_d[:, :], in_=stats[:])

    _split_multiwait(nc)
    return nc


def kernel(logits, labels, gate_logits, expert_indices):
    global _nc_cache, _last_results
    logits = np.asarray(logits, dtype=np.float32).reshape(NT, V)
    labels = np.asarray(labels).reshape(NT).astype(np.int64)
    gate_logits = np.asarray(gate_logits, dtype=np.float32).reshape(NT, E)
    expert_indices = np.asarray(expert_indices).reshape(NT, K).astype(np.int64)

    if _nc_cache is None:
        _nc_cache = _build()
    nc = _nc_cache

    tok = np.arange(TPC, dtype=np.int64)
    in_maps = []
    for c in range(N_CORES):
        sl = slice(c * TPC, (c + 1) * TPC)
        lab = labels[sl]
        off = (tok * V + lab).astype(np.int32)
        in_maps.append(
            {
                "logits": logits[sl],
                "goff": np.ascontiguousarray(off.reshape(NB, P).T),
                "labf": np.ascontiguousarray(
                    lab.reshape(NB, P).T.astype(np.float32)
                ),
                "gate": np.ascontiguousarray(
                    gate_logits[sl].reshape(NB, P, E).transpose(1, 0, 2).reshape(P, NB * E)
                ),
                "eidx": np.ascontiguousarray(
                    expert_indices[sl].reshape(NB, P, K).transpose(1, 0, 2)
                    .reshape(P, NB * K).astype(np.float32)
                ),
            }
        )

    res = run_bass_kernel_spmd(nc, in_maps, core_ids=list(range(N_CORES)))
    _last_results = res

    st = np.stack([np.asarray(res.results[c]["stats"]) for c in range(N_CORES)])
    st = st.astype(np.float64)
    ce_sum = st[:, :, 0:8].sum()
    valid_count = st[:, :, 8:16].sum()
    load = st[:, :, 16:24].sum(axis=(0, 1))
    counts = st[:, :, 24:32].sum(axis=(0, 1))

    base_loss = ce_sum / max(valid_count, 1.0)
    aux_loss = ((counts - counts.mean()) ** 2).mean()
    lb_loss = ((load - load.mean()) ** 2).mean()
    return np.array(base_loss + AUX_W * aux_loss + LB_W * lb_loss, dtype=np.float32)
